# revision 1
# baseline (speedup 1.0000x reference)
"""Fused multi-head attention (B=2, T=2048, D=2048, H=16) on 8 trn2 NeuronCores.

Sharding: core c handles batch b=c//4 and heads [4g, 4g+4), g=c%4 (tensor
parallel over heads x data parallel over batch). Each core computes its
4 heads' contribution to out[b] = attn(x[b]) @ Wo^T; the host sums the 4
partials per batch.

Device algorithm (all matmuls fp32r, PSUM fp32):
  P1a  qT = (Wq_s/sqrt(dh)) @ x^T, kT = Wk_s @ x^T       [E=512, T]
  P1b  v  = x @ Wv_s^T                                    [T, E]
  P2   per i-chunk (512 queries), per head:
         S^T[j,i] = kT_h^T-contract : PSUM -> exp (ACT) -> * exp(mask^T)
         ctx^T[e,i] += v_h^T @ P^T  (PSUM, 16 j-tiles)
         l[i]      += 1^T @ P^T     (PSUM)
         ctx^T *= broadcast(1/l)    (outer-product bcast + DVE mul)
  P3   out[t,dd] = sum_e ctx^T[e,t] * WoT[e,dd]  -> DRAM

Inputs are pre-transposed/sharded/scaled on the host so every device matmul
is a natural [K=128-tile] x [N<=512] fp32r op.
"""

import os

import numpy as np

import concourse.bass as bass
import concourse.mybir as mybir
import concourse.tile as tile
from concourse import bacc
from concourse.bass_utils import run_bass_kernel_spmd

F32 = mybir.dt.float32
F32R = mybir.dt.float32r
EXP = mybir.ActivationFunctionType.Exp

B, T, D, H = 2, 2048, 2048, 16
DH = D // H          # 128
E = 512              # features per core (4 heads)
HPC = 4              # heads per core
NT = T // 128        # 16 token tiles
ND = D // 128        # 16 model-dim tiles
NE = E // 128        # 4 e-tiles per core
NI = T // 512        # 4 i-chunks (moving dim)
NJ = NT              # 16 j-tiles

_NC_CACHE = {}

# per-(jt, ic) mask-block class: 0 = fully masked (skip), 1 = unmasked
# (skip the mask multiply), 2 = mixed (apply exp(mask) elementwise)
SKIP, NOMULT, MIXED = 0, 1, 2


def _build(cls_key):
    cls = np.asarray(cls_key, dtype=np.int64).reshape(NJ, NI)
    nc = bacc.Bacc(None, target_bir_lowering=False, debug=False)
    xt = nc.declare_dram_parameter("xt", [D, T], F32R, isOutput=False)
    wq = nc.declare_dram_parameter("wq", [D, E], F32R, isOutput=False)
    wk = nc.declare_dram_parameter("wk", [D, E], F32R, isOutput=False)
    wv = nc.declare_dram_parameter("wv", [D, E], F32R, isOutput=False)
    wo = nc.declare_dram_parameter("wo", [E, D], F32R, isOutput=False)
    em = nc.declare_dram_parameter("em", [T, T], F32, isOutput=False)
    onk = nc.declare_dram_parameter("onk", [128, 1], F32R, isOutput=False)
    onp = nc.declare_dram_parameter("onp", [1, 128], F32R, isOutput=False)
    idn = nc.declare_dram_parameter("idn", [128, 128], F32R, isOutput=False)
    out = nc.declare_dram_parameter("out", [T, D], F32, isOutput=True)

    with tile.TileContext(nc) as tc:
        # ---- long-lived residents (stack order: ctx outlives qk/v) -----
        pool_ctx = tc.alloc_tile_pool(name="res_ctx", bufs=1)
        ctx = [pool_ctx.tile([128, T], F32R, name=f"ctx{m}") for m in range(NE)]
        pool_v = tc.alloc_tile_pool(name="res_v", bufs=1)
        v_sb = pool_v.tile([128, NT, E], F32R)
        pool_qk = tc.alloc_tile_pool(name="res_qk", bufs=1)
        qT = [pool_qk.tile([128, T], F32R, name=f"qT{m}") for m in range(NE)]
        kT = [pool_qk.tile([128, T], F32R, name=f"kT{m}") for m in range(NE)]

        scope_p1a = nc.named_scope("P1a_qk"); scope_p1a.__enter__()
        # ---- P1a: q/k projections --------------------------------------
        p_w = tc.alloc_tile_pool(name="p1w", bufs=1)
        wq_sb = p_w.tile([128, ND, E], F32R)
        wk_sb = p_w.tile([128, ND, E], F32R)
        for dt in range(ND):
            nc.sync.dma_start(out=wq_sb[:, dt, :], in_=wq.ap()[dt * 128:(dt + 1) * 128, :])
            nc.sync.dma_start(out=wk_sb[:, dt, :], in_=wk.ap()[dt * 128:(dt + 1) * 128, :])
        p_x = tc.alloc_tile_pool(name="p1x", bufs=3)
        p_ps1 = tc.alloc_tile_pool(name="p1ps", bufs=8, space="PSUM")
        for nch in range(NI):
            psq, psk = {}, {}
            for m in range(NE):
                ps_q = p_ps1.tile([128, 512], F32, name="ps_q", bufs=4)
                ps_k = p_ps1.tile([128, 512], F32, name="ps_k", bufs=4)
                psq[m], psk[m] = ps_q, ps_k
            for dt in range(ND):
                xtile = p_x.tile([128, 512], F32R, name="xtile")
                nc.sync.dma_start(
                    out=xtile,
                    in_=xt.ap()[dt * 128:(dt + 1) * 128, nch * 512:(nch + 1) * 512])
                st, sp = dt == 0, dt == ND - 1
                for m in range(NE):
                    nc.tensor.matmul(psq[m], wq_sb[:, dt, m * 128:(m + 1) * 128],
                                     xtile, start=st, stop=sp)
                    nc.tensor.matmul(psk[m], wk_sb[:, dt, m * 128:(m + 1) * 128],
                                     xtile, start=st, stop=sp)
            for m in range(NE):
                nc.scalar.copy(qT[m][:, nch * 512:(nch + 1) * 512], psq[m])
                nc.vector.tensor_copy(kT[m][:, nch * 512:(nch + 1) * 512], psk[m])
        p_ps1.release()
        p_x.release()
        p_w.release()
        scope_p1a.__exit__(None, None, None)
        scope_p1b = nc.named_scope("P1b_v"); scope_p1b.__enter__()

        # ---- P1b: v via vT = Wv_s @ x^T, then PE-transpose -------------
        p_wv = tc.alloc_tile_pool(name="p1bw", bufs=1)
        wv_sb = p_wv.tile([128, ND, E], F32R)
        idn_sb = p_wv.tile([128, 128], F32R)
        nc.sync.dma_start(out=idn_sb, in_=idn.ap())
        for dt in range(ND):
            nc.sync.dma_start(out=wv_sb[:, dt, :], in_=wv.ap()[dt * 128:(dt + 1) * 128, :])
        p_vt = tc.alloc_tile_pool(name="p1bvt", bufs=2)
        p_x2 = tc.alloc_tile_pool(name="p1bx", bufs=3)
        p_ps2 = tc.alloc_tile_pool(name="p1bps", bufs=4, space="PSUM")
        p_pst = tc.alloc_tile_pool(name="p1bpst", bufs=4, space="PSUM")
        for nch in range(NI):
            psv = {}
            for m in range(NE):
                ps_v = p_ps2.tile([128, 512], F32, name="ps_v", bufs=4)
                psv[m] = ps_v
            for dt in range(ND):
                xtile2 = p_x2.tile([128, 512], F32R, name="xtile2")
                nc.sync.dma_start(
                    out=xtile2,
                    in_=xt.ap()[dt * 128:(dt + 1) * 128, nch * 512:(nch + 1) * 512])
                for m in range(NE):
                    nc.tensor.matmul(psv[m], wv_sb[:, dt, m * 128:(m + 1) * 128],
                                     xtile2, start=(dt == 0), stop=(dt == ND - 1))
            vtc = p_vt.tile([128, NE, 512], F32R, name="vtc")
            for m in range(NE):
                nc.scalar.copy(vtc[:, m, :], psv[m])
            for m in range(NE):
                for tl in range(4):
                    ps_t = p_pst.tile([128, 128], F32R, name="ps_t")
                    nc.tensor.transpose(
                        ps_t, vtc[:, m, tl * 128:(tl + 1) * 128], idn_sb)
                    nc.vector.tensor_copy(
                        v_sb[:, nch * 4 + tl, m * 128:(m + 1) * 128], ps_t)
        p_pst.release()
        p_ps2.release()
        p_x2.release()
        p_vt.release()
        p_wv.release()
        scope_p1b.__exit__(None, None, None)
        scope_p2 = nc.named_scope("P2_attn"); scope_p2.__enter__()

        # ---- P2: attention ---------------------------------------------
        p_const = tc.alloc_tile_pool(name="p2c", bufs=1)
        ones_k = p_const.tile([128, 1], F32R)
        ones_p = p_const.tile([1, 128], F32R)
        nc.sync.dma_start(out=ones_k, in_=onk.ap())
        nc.sync.dma_start(out=ones_p, in_=onp.ap())

        p_em = tc.alloc_tile_pool(name="p2em", bufs=3)
        p_pt = tc.alloc_tile_pool(name="p2pt", bufs=3)
        p_ptm = tc.alloc_tile_pool(name="p2ptm", bufs=3)
        p_bs = tc.alloc_tile_pool(name="p2bs", bufs=2)
        p_rr = tc.alloc_tile_pool(name="p2rr", bufs=2)
        ps_ctx_pool = tc.alloc_tile_pool(name="p2psc", bufs=2, space="PSUM")
        ps_l_pool = tc.alloc_tile_pool(name="p2psl", bufs=2, space="PSUM")
        ps_s_pool = tc.alloc_tile_pool(name="p2pss", bufs=3, space="PSUM")
        ps_b_pool = tc.alloc_tile_pool(name="p2psb", bufs=1, space="PSUM")

        for ic in range(NI):
            isl = slice(ic * 512, (ic + 1) * 512)
            surv = [jt for jt in range(NJ) if cls[jt, ic] != SKIP]
            assert surv, f"i-chunk {ic}: every key block masked"
            first, last = surv[0], surv[-1]
            for hp in range(HPC // 2):
                heads = (2 * hp, 2 * hp + 1)
                cps, lps = {}, {}
                for h in heads:
                    ps_c = ps_ctx_pool.tile([128, 512], F32, name="ps_c")
                    ps_l = ps_l_pool.tile([1, 512], F32, name="ps_l")
                    cps[h], lps[h] = ps_c, ps_l
                for jt in surv:
                    if cls[jt, ic] == MIXED:
                        emt = p_em.tile([128, 512], F32, name="emt")
                        nc.sync.dma_start(
                            out=emt, in_=em.ap()[jt * 128:(jt + 1) * 128, isl])
                    for h in heads:
                        ps_s = ps_s_pool.tile([128, 512], F32, name="ps_s")
                        nc.tensor.matmul(
                            ps_s, kT[h][:, jt * 128:(jt + 1) * 128],
                            qT[h][:, isl], start=True, stop=True)
                        pt = p_pt.tile([128, 512], F32R, name="pt")
                        nc.scalar.activation(pt, ps_s, EXP)
                        if cls[jt, ic] == MIXED:
                            ptm = p_ptm.tile([128, 512], F32R, name="ptm")
                            nc.vector.tensor_mul(ptm, pt, emt)
                        else:
                            ptm = pt
                        st, sp = jt == first, jt == last
                        nc.tensor.matmul(
                            cps[h], v_sb[:, jt, h * 128:(h + 1) * 128],
                            ptm, start=st, stop=sp)
                        nc.tensor.matmul(lps[h], ones_k, ptm,
                                         start=st, stop=sp)
                for h in heads:
                    rr = p_rr.tile([1, 512], F32R, name="rr")
                    with nc.allow_low_precision(reason="softmax recip f32r"):
                        nc.vector.reciprocal(rr, lps[h])
                    ps_b = ps_b_pool.tile([128, 512], F32, name="ps_b")
                    nc.tensor.matmul(ps_b, ones_p, rr, start=True, stop=True)
                    bsb = p_bs.tile([128, 512], F32, name="bsb")
                    nc.scalar.copy(bsb, ps_b)
                    nc.vector.tensor_mul(ctx[h][:, isl], cps[h], bsb)
        for p in (ps_b_pool, ps_s_pool, ps_l_pool, ps_ctx_pool,
                  p_rr, p_bs, p_ptm, p_pt, p_em, p_const):
            p.release()
        pool_qk.release()
        pool_v.release()
        scope_p2.__exit__(None, None, None)
        scope_p3 = nc.named_scope("P3_out"); scope_p3.__enter__()

        # ---- P3: output projection -------------------------------------
        p_wo = tc.alloc_tile_pool(name="p3w", bufs=1)
        wo_sb = p_wo.tile([128, NE, D], F32R)
        for et in range(NE):
            nc.sync.dma_start(out=wo_sb[:, et, :], in_=wo.ap()[et * 128:(et + 1) * 128, :])
        p_ot = tc.alloc_tile_pool(name="p3o", bufs=3)
        p_ps3 = tc.alloc_tile_pool(name="p3ps", bufs=3, space="PSUM")
        for tt in range(NT):
            tsl = slice(tt * 128, (tt + 1) * 128)
            for nch in range(NI):
                ps_o = p_ps3.tile([128, 512], F32, name="ps_o")
                for et in range(NE):
                    nc.tensor.matmul(
                        ps_o, ctx[et][:, tsl],
                        wo_sb[:, et, nch * 512:(nch + 1) * 512],
                        start=(et == 0), stop=(et == NE - 1))
                ot = p_ot.tile([128, 512], F32, name="ot")
                nc.scalar.copy(ot, ps_o)
                nc.sync.dma_start(
                    out=out.ap()[tsl, nch * 512:(nch + 1) * 512], in_=ot)
        p_ps3.release()
        p_ot.release()
        p_wo.release()
        pool_ctx.release()
        scope_p3.__exit__(None, None, None)

    nc.compile()
    return nc


def _get_nc(cls_key):
    if cls_key not in _NC_CACHE:
        _NC_CACHE[cls_key] = _build(cls_key)
    return _NC_CACHE[cls_key]


def kernel(x, Wq, Wk, Wv, Wo, attn_mask):
    x = np.asarray(x, dtype=np.float32)
    Wq = np.asarray(Wq, dtype=np.float32)
    Wk = np.asarray(Wk, dtype=np.float32)
    Wv = np.asarray(Wv, dtype=np.float32)
    Wo = np.asarray(Wo, dtype=np.float32)
    mask = np.asarray(attn_mask, dtype=np.float32).reshape(T, T)

    emT = np.ascontiguousarray(np.exp(mask).T)
    xT = [np.ascontiguousarray(x[b].T) for b in range(B)]
    scale = np.float32(1.0 / np.sqrt(DH))

    blocks = emT.reshape(NJ, 128, NI, 512)
    cls = np.full((NJ, NI), MIXED, dtype=np.int64)
    for jt in range(NJ):
        for ic in range(NI):
            sub = blocks[jt, :, ic, :]
            if not sub.any():
                cls[jt, ic] = SKIP
            elif np.all(sub == 1.0):
                cls[jt, ic] = NOMULT
    cls_key = tuple(cls.flatten().tolist())

    in_maps = []
    for c in range(8):
        b, g = c // 4, c % 4
        rows = slice(E * g, E * (g + 1))
        in_maps.append({
            "xt": xT[b],
            "wq": np.ascontiguousarray((Wq[rows, :] * scale).T),
            "wk": np.ascontiguousarray(Wk[rows, :].T),
            "wv": np.ascontiguousarray(Wv[rows, :].T),
            "wo": np.ascontiguousarray(Wo[:, rows].T),
            "em": emT,
            "onk": np.ones((128, 1), dtype=np.float32),
            "onp": np.ones((1, 128), dtype=np.float32),
            "idn": np.eye(128, dtype=np.float32),
        })

    global _LAST_IN_MAPS, _LAST_NC
    _LAST_IN_MAPS = in_maps
    nc = _get_nc(cls_key)
    _LAST_NC = nc
    res = run_bass_kernel_spmd(nc, in_maps, list(range(8)))
    outs = [r["out"] for r in res.results]
    full = np.stack([
        outs[0] + outs[1] + outs[2] + outs[3],
        outs[4] + outs[5] + outs[6] + outs[7],
    ]).astype(np.float32)
    return full



# revision 4
# speedup vs baseline: 1.2825x; 1.2825x over previous
"""Fused multi-head attention (B=2, T=2048, D=2048, H=16) on 8 trn2 NeuronCores.

Sharding: core c handles batch b=c//4 and heads [4g, 4g+4), g=c%4 (tensor
parallel over heads x data parallel over batch). Each core computes its
4 heads' contribution to out[b] = attn(x[b]) @ Wo^T; the host sums the 4
partials per batch.

v2: all matmul operands fp16 (PSUM stays fp32), x^T DMA'd once into
resident SBUF, weights prefetched at kernel start, v computed directly in
[token, feature] layout (x-stationary matmuls, no PE transposes), exp
batched over j-tile pairs ([128,1024] ACT calls), causal diagonal masks as
2 resident [128,2,512] pattern tiles, software-pipelined attention inner
loop sized to exactly 8 PSUM banks, fp16 output.

Device algorithm (per core, E=512 features = 4 heads):
  P1  qT/kT = (W_s) @ x^T   [E rows as 4x(dh=128), T]   (Wq pre-scaled)
      v     = x @ Wv_s^T    [T, E]
  P2  per i-chunk (512 q), head pair: S^T pair = kT_jt^T-contract @ qT
        -> exp (ACT, [128,1024]) -> *mask (diag pairs) ->
        ctx^T += v_jt^T @ P^T ; l += 1^T @ P^T ; ctx^T *= bcast(1/l)
  P3  out[t, d] = sum_e ctx^T[e, t] * WoT[e, d] -> DRAM (fp16)
"""

import numpy as np

import concourse.bass as bass
import concourse.mybir as mybir
import concourse.tile as tile
from concourse import bacc
from concourse.bass_utils import run_bass_kernel_spmd

F32 = mybir.dt.float32
F16 = mybir.dt.float16
EXP = mybir.ActivationFunctionType.Exp

B, T, D, H = 2, 2048, 2048, 16
DH = D // H          # 128
E = 512              # features per core (4 heads)
HPC = 4              # heads per core
NT = T // 128        # 16 token tiles
ND = D // 128        # 16 model-dim tiles
NE = E // 128        # 4 e-tiles per core
NI = T // 512        # 4 i-chunks (query chunks)
NJ = NT              # 16 j-tiles (key tiles)
NCH = T // 1024      # 2 big token chunks for the projections

_NC_CACHE = {}

# per-(jt, ic) mask-block class: 0 = fully masked (skip), 1 = unmasked
# (skip the mask multiply), 2 = mixed (multiply by exp(mask) elementwise)
SKIP, NOMULT, MIXED = 0, 1, 2


def _build(cls_key, causal):
    cls = np.asarray(cls_key, dtype=np.int64).reshape(NJ, NI)
    nc = bacc.Bacc(None, target_bir_lowering=False, debug=False)
    xt = nc.declare_dram_parameter("xt", [D, T], F16, isOutput=False)
    wq = nc.declare_dram_parameter("wq", [D, E], F16, isOutput=False)
    wk = nc.declare_dram_parameter("wk", [D, E], F16, isOutput=False)
    wv = nc.declare_dram_parameter("wv", [D, E], F16, isOutput=False)
    wo = nc.declare_dram_parameter("wo", [E, D], F16, isOutput=False)
    if causal:
        em2 = nc.declare_dram_parameter("em2", [128, 2, 2, 512], F16, isOutput=False)
    else:
        em = nc.declare_dram_parameter("em", [T, T], F16, isOutput=False)
    onk = nc.declare_dram_parameter("onk", [128, 1], F16, isOutput=False)
    onp = nc.declare_dram_parameter("onp", [1, 128], F16, isOutput=False)
    out = nc.declare_dram_parameter("out", [T, D], F16, isOutput=True)

    with tile.TileContext(nc) as tc:
        # ---- long-lived residents --------------------------------------
        pool_res = tc.alloc_tile_pool(name="res", bufs=1)
        ctx = [pool_res.tile([128, T], F16, name=f"ctx{m}") for m in range(NE)]
        v_sb = pool_res.tile([128, NT, E], F16)
        wo_sb = pool_res.tile([128, NE, D], F16)
        ones_k = pool_res.tile([128, 1], F16)
        ones_p = pool_res.tile([1, 128], F16)
        scratch = pool_res.tile([1, 8], F16)
        if causal:
            em_sb = pool_res.tile([128, 2, 2, 512], F16)

        pool_qk = tc.alloc_tile_pool(name="res_qk", bufs=1)
        qT = [pool_qk.tile([128, T], F16, name=f"qT{m}") for m in range(NE)]
        kT = [pool_qk.tile([128, T], F16, name=f"kT{m}") for m in range(NE)]

        pool_p1 = tc.alloc_tile_pool(name="p1", bufs=1)
        xt_sb = pool_p1.tile([128, ND, T], F16)
        wq_sb = pool_p1.tile([128, ND, E], F16)
        wk_sb = pool_p1.tile([128, ND, E], F16)
        wv_sb = pool_p1.tile([128, ND, E], F16)

        # ---- DMA schedule: tiny consts, then wq+x(tch0) interleaved so
        # the first projection pass can ride the stream, then the rest.
        nc.sync.dma_start(out=ones_k, in_=onk.ap())
        nc.sync.dma_start(out=ones_p, in_=onp.ap())
        if causal:
            nc.sync.dma_start(out=em_sb, in_=em2.ap())
        for dt in range(ND):
            nc.sync.dma_start(out=wq_sb[:, dt, :], in_=wq.ap()[dt * 128:(dt + 1) * 128, :])
            nc.sync.dma_start(
                out=xt_sb[:, dt, 0:1024],
                in_=xt.ap()[dt * 128:(dt + 1) * 128, 0:1024])
        for dt in range(ND):
            nc.sync.dma_start(out=wk_sb[:, dt, :], in_=wk.ap()[dt * 128:(dt + 1) * 128, :])
        for dt in range(ND):
            nc.sync.dma_start(out=wv_sb[:, dt, :], in_=wv.ap()[dt * 128:(dt + 1) * 128, :])
        for dt in range(ND):
            nc.sync.dma_start(
                out=xt_sb[:, dt, 1024:2048],
                in_=xt.ap()[dt * 128:(dt + 1) * 128, 1024:2048])
        for et in range(NE):
            nc.sync.dma_start(out=wo_sb[:, et, :], in_=wo.ap()[et * 128:(et + 1) * 128, :])

        # warm the ACT exp table set before P2 needs it
        nc.scalar.activation(scratch[0:1, 0:1], ones_k[0:1, 0:1], EXP)

        scope_p1 = nc.named_scope("P1_qkv"); scope_p1.__enter__()
        # ---- P1: q/k (feature-major) and v (token-major) projections ----
        p_psqk = tc.alloc_tile_pool(name="p1psqk", bufs=2, space="PSUM")
        p_psv = tc.alloc_tile_pool(name="p1psv", bufs=3, space="PSUM")
        for tch in range(NCH):
            for ti, (w_sb, dst) in enumerate(((wq_sb, qT), (wk_sb, kT))):
                for m in range(NE):
                    for half in range(2):
                        tsl = slice(tch * 1024 + half * 512,
                                    tch * 1024 + (half + 1) * 512)
                        ps = p_psqk.tile([128, 512], F32, name="ps_qk")
                        for dt in range(ND):
                            nc.tensor.matmul(ps, w_sb[:, dt, m * 128:(m + 1) * 128],
                                             xt_sb[:, dt, tsl],
                                             start=(dt == 0), stop=(dt == ND - 1))
                        if (m + ti + half) % 2 == 0:
                            nc.scalar.copy(dst[m][:, tsl], ps)
                        else:
                            nc.vector.tensor_copy(dst[m][:, tsl], ps)
            for tb in range(8):
                tbg = tch * 8 + tb
                ps = p_psv.tile([128, 512], F32, name="ps_v")
                for dt in range(ND):
                    nc.tensor.matmul(
                        ps, xt_sb[:, dt, tbg * 128:(tbg + 1) * 128],
                        wv_sb[:, dt, :], start=(dt == 0), stop=(dt == ND - 1))
                if tb % 2 == 0:
                    nc.vector.tensor_copy(v_sb[:, tbg, :], ps)
                else:
                    nc.scalar.copy(v_sb[:, tbg, :], ps)
        p_psv.release()
        p_psqk.release()
        pool_p1.release()
        scope_p1.__exit__(None, None, None)

        scope_p2 = nc.named_scope("P2_attn"); scope_p2.__enter__()
        # ---- P2: attention ---------------------------------------------
        p_pt = tc.alloc_tile_pool(name="p2pt", bufs=3)
        p_em = tc.alloc_tile_pool(name="p2em", bufs=3)
        p_bs = tc.alloc_tile_pool(name="p2bs", bufs=2)
        p_rr = tc.alloc_tile_pool(name="p2rr", bufs=2)
        ps_s_pool = tc.alloc_tile_pool(name="p2pss", bufs=2, space="PSUM")
        ps_c_pool = tc.alloc_tile_pool(name="p2psc", bufs=2, space="PSUM")
        ps_lb_pool = tc.alloc_tile_pool(name="p2pslb", bufs=2, space="PSUM")

        for ic in range(NI):
            isl = slice(ic * 512, (ic + 1) * 512)
            surv = [jt for jt in range(NJ) if cls[jt, ic] != SKIP]
            assert surv, f"i-chunk {ic}: every key block masked"
            first, last = surv[0], surv[-1]
            pairs = [tuple(surv[i:i + 2]) for i in range(0, len(surv), 2)]
            for hp in range(HPC // 2):
                heads = (2 * hp, 2 * hp + 1)
                cps, lps = {}, {}
                for h in heads:
                    cps[h] = ps_c_pool.tile([128, 512], F32, name="ps_c", tag="c")
                    lps[h] = ps_lb_pool.tile([1, 512], F32, name="ps_l", tag="lb")
                prev = None
                for pr in pairs:
                    # mask tiles for this pair (None / resident slice / DMA'd)
                    emop = None
                    if causal and cls[pr[0], ic] == MIXED:
                        # both halves of a MIXED pair are MIXED in the causal
                        # layout; two resident pattern pairs cover o=0..3
                        kk = (pr[0] - 4 * ic) // 2
                        emop = em_sb[:, kk, :, :]
                    elif not causal and any(cls[jt, ic] == MIXED for jt in pr):
                        emt = p_em.tile([128, 2, 512], F16, name="emt")
                        for j, jt in enumerate(pr):
                            if cls[jt, ic] == MIXED:
                                nc.sync.dma_start(
                                    out=emt[:, j, :],
                                    in_=em.ap()[jt * 128:(jt + 1) * 128, isl])
                            else:
                                nc.vector.memset(emt[:, j, :], 1.0)
                        emop = emt
                    pts = {}
                    for h in heads:
                        ps_s = ps_s_pool.tile([128, 2, 512], F32, name="ps_s")
                        for j, jt in enumerate(pr):
                            nc.tensor.matmul(
                                ps_s[:, j, :], kT[h][:, jt * 128:(jt + 1) * 128],
                                qT[h][:, isl], start=True, stop=True)
                        pt = p_pt.tile([128, 2, 512], F16, name="pt")
                        if len(pr) == 2:
                            nc.scalar.activation(pt, ps_s, EXP)
                        else:
                            nc.scalar.activation(pt[:, 0, :], ps_s[:, 0, :], EXP)
                        if emop is not None:
                            if len(pr) == 2:
                                nc.vector.tensor_mul(pt, pt, emop)
                            else:
                                nc.vector.tensor_mul(
                                    pt[:, 0, :], pt[:, 0, :], emop[:, 0, :])
                        pts[h] = pt
                    if prev is not None:
                        ppr, ppts = prev
                        for h in heads:
                            for j, jt in enumerate(ppr):
                                st, sp = jt == first, jt == last
                                nc.tensor.matmul(
                                    cps[h], v_sb[:, jt, h * 128:(h + 1) * 128],
                                    ppts[h][:, j, :], start=st, stop=sp)
                                nc.tensor.matmul(lps[h], ones_k,
                                                 ppts[h][:, j, :],
                                                 start=st, stop=sp)
                    prev = (pr, pts)
                ppr, ppts = prev
                for h in heads:
                    for j, jt in enumerate(ppr):
                        st, sp = jt == first, jt == last
                        nc.tensor.matmul(
                            cps[h], v_sb[:, jt, h * 128:(h + 1) * 128],
                            ppts[h][:, j, :], start=st, stop=sp)
                        nc.tensor.matmul(lps[h], ones_k, ppts[h][:, j, :],
                                         start=st, stop=sp)
                for h in heads:
                    rr = p_rr.tile([1, 512], F16, name="rr")
                    with nc.allow_low_precision(reason="softmax recip f16"):
                        nc.vector.reciprocal(rr, lps[h])
                    ps_b = ps_lb_pool.tile([128, 512], F32, name="ps_b", tag="lb")
                    nc.tensor.matmul(ps_b, ones_p, rr, start=True, stop=True)
                    bsb = p_bs.tile([128, 512], F16, name="bsb")
                    nc.vector.tensor_copy(bsb, ps_b)
                    nc.vector.tensor_mul(ctx[h][:, isl], cps[h], bsb)
        for p in (ps_lb_pool, ps_c_pool, ps_s_pool, p_rr, p_bs, p_em, p_pt):
            p.release()
        pool_qk.release()
        scope_p2.__exit__(None, None, None)

        scope_p3 = nc.named_scope("P3_out"); scope_p3.__enter__()
        # ---- P3: output projection -------------------------------------
        p_ot = tc.alloc_tile_pool(name="p3o", bufs=3)
        p_ps3 = tc.alloc_tile_pool(name="p3ps", bufs=2, space="PSUM")
        for tt in range(NT):
            tsl = slice(tt * 128, (tt + 1) * 128)
            for nch in range(NI):
                ps_o = p_ps3.tile([128, 512], F32, name="ps_o")
                for et in range(NE):
                    nc.tensor.matmul(
                        ps_o, ctx[et][:, tsl],
                        wo_sb[:, et, nch * 512:(nch + 1) * 512],
                        start=(et == 0), stop=(et == NE - 1))
                ot = p_ot.tile([128, 512], F16, name="ot")
                if (tt + nch) % 2 == 0:
                    nc.scalar.copy(ot, ps_o)
                else:
                    nc.vector.tensor_copy(ot, ps_o)
                nc.sync.dma_start(
                    out=out.ap()[tsl, nch * 512:(nch + 1) * 512], in_=ot)
        p_ps3.release()
        p_ot.release()
        pool_res.release()
        scope_p3.__exit__(None, None, None)

    nc.compile()
    return nc


def _get_nc(cls_key, causal):
    key = (cls_key, causal)
    if key not in _NC_CACHE:
        _NC_CACHE[key] = _build(cls_key, causal)
    return _NC_CACHE[key]


def _causal_pattern(o):
    p = np.arange(128)[:, None]
    f = np.arange(512)[None, :]
    return (p + o * 128 <= f).astype(np.float16)


def kernel(x, Wq, Wk, Wv, Wo, attn_mask):
    x = np.asarray(x, dtype=np.float32)
    Wq = np.asarray(Wq, dtype=np.float32)
    Wk = np.asarray(Wk, dtype=np.float32)
    Wv = np.asarray(Wv, dtype=np.float32)
    Wo = np.asarray(Wo, dtype=np.float32)
    mask = np.asarray(attn_mask, dtype=np.float32).reshape(T, T)

    emT = np.ascontiguousarray(np.exp(mask).T)
    scale = np.float32(1.0 / np.sqrt(DH))

    blocks = emT.reshape(NJ, 128, NI, 512)
    cls = np.full((NJ, NI), MIXED, dtype=np.int64)
    for jt in range(NJ):
        for ic in range(NI):
            sub = blocks[jt, :, ic, :]
            if not sub.any():
                cls[jt, ic] = SKIP
            elif np.all(sub == 1.0):
                cls[jt, ic] = NOMULT
    cls_key = tuple(cls.flatten().tolist())

    # causal fast path: survivors are a prefix, MIXED blocks are the last 4
    # of each i-chunk and match the canonical diagonal patterns
    causal = True
    pat = [_causal_pattern(o).astype(np.float32) for o in range(4)]
    for ic in range(NI):
        surv = [jt for jt in range(NJ) if cls[jt, ic] != SKIP]
        mix = [jt for jt in range(NJ) if cls[jt, ic] == MIXED]
        if surv != list(range(4 * ic + 4)) or mix != list(range(4 * ic, 4 * ic + 4)):
            causal = False
            break
        for jt in mix:
            if not np.array_equal(blocks[jt, :, ic, :], pat[jt - 4 * ic]):
                causal = False
                break
        if not causal:
            break

    em2 = np.zeros((128, 2, 2, 512), dtype=np.float16)
    for kk in range(2):
        for j in range(2):
            em2[:, kk, j, :] = _causal_pattern(2 * kk + j)

    xT = [np.ascontiguousarray(x[b].T).astype(np.float16) for b in range(B)]
    emT16 = emT.astype(np.float16)

    in_maps = []
    for c in range(8):
        b, g = c // 4, c % 4
        rows = slice(E * g, E * (g + 1))
        m = {
            "xt": xT[b],
            "wq": np.ascontiguousarray((Wq[rows, :] * scale).T).astype(np.float16),
            "wk": np.ascontiguousarray(Wk[rows, :].T).astype(np.float16),
            "wv": np.ascontiguousarray(Wv[rows, :].T).astype(np.float16),
            "wo": np.ascontiguousarray(Wo[:, rows].T).astype(np.float16),
            "onk": np.ones((128, 1), dtype=np.float16),
            "onp": np.ones((1, 128), dtype=np.float16),
        }
        if causal:
            m["em2"] = em2
        else:
            m["em"] = emT16
        in_maps.append(m)

    global _LAST_IN_MAPS, _LAST_NC
    _LAST_IN_MAPS = in_maps
    nc = _get_nc(cls_key, causal)
    _LAST_NC = nc
    res = run_bass_kernel_spmd(nc, in_maps, list(range(8)))
    outs = [r["out"].astype(np.float32) for r in res.results]
    full = np.stack([
        outs[0] + outs[1] + outs[2] + outs[3],
        outs[4] + outs[5] + outs[6] + outs[7],
    ]).astype(np.float32)
    return full


# revision 10
# speedup vs baseline: 1.3851x; 1.0800x over previous
"""Fused multi-head attention (B=2, T=2048, D=2048, H=16) on 8 trn2 NeuronCores.

Sharding: core c handles batch b=c//4 and heads [4g, 4g+4), g=c%4 (tensor
parallel over heads x data parallel over batch). Each core computes its
4 heads' contribution to out[b] = attn(x[b]) @ Wo^T; the host sums the 4
partials per batch.

v2: all matmul operands fp16 (PSUM stays fp32), x^T DMA'd once into
resident SBUF, weights prefetched at kernel start, v computed directly in
[token, feature] layout (x-stationary matmuls, no PE transposes), exp
batched over j-tile pairs ([128,1024] ACT calls), causal diagonal masks as
2 resident [128,2,512] pattern tiles, software-pipelined attention inner
loop sized to exactly 8 PSUM banks, fp16 output.

Device algorithm (per core, E=512 features = 4 heads):
  P1  qT/kT = (W_s) @ x^T   [E rows as 4x(dh=128), T]   (Wq pre-scaled)
      v     = x @ Wv_s^T    [T, E]
  P2  per i-chunk (512 q), head pair: S^T pair = kT_jt^T-contract @ qT
        -> exp (ACT, [128,1024]) -> *mask (diag pairs) ->
        ctx^T += v_jt^T @ P^T ; L += ones128^T @ P^T (l replicated on all
        128 partitions, same PE tile config as ctx, no broadcast needed);
        ctx^T *= recip(L).  Upper-diagonal pairs trimmed to 256 queries.
  P3  out[t, d] = sum_e ctx^T[e, t] * WoT[e, d] -> DRAM (fp16)
"""

import numpy as np

import concourse.bass as bass
import concourse.mybir as mybir
import concourse.tile as tile
from concourse import bacc
from concourse.bass_utils import run_bass_kernel_spmd

F32 = mybir.dt.float32
F16 = mybir.dt.float16
EXP = mybir.ActivationFunctionType.Exp

B, T, D, H = 2, 2048, 2048, 16
DH = D // H          # 128
E = 512              # features per core (4 heads)
HPC = 4              # heads per core
NT = T // 128        # 16 token tiles
ND = D // 128        # 16 model-dim tiles
NE = E // 128        # 4 e-tiles per core
NI = T // 512        # 4 i-chunks (query chunks)
NJ = NT              # 16 j-tiles (key tiles)
NCH = T // 1024      # 2 big token chunks for the projections

_NC_CACHE = {}

# per-(jt, ic) mask-block class: 0 = fully masked (skip), 1 = unmasked
# (skip the mask multiply), 2 = mixed (multiply by exp(mask) elementwise)
SKIP, NOMULT, MIXED = 0, 1, 2


def _build(cls_key, causal):
    cls = np.asarray(cls_key, dtype=np.int64).reshape(NJ, NI)
    nc = bacc.Bacc(None, target_bir_lowering=False, debug=False)
    xt = nc.declare_dram_parameter("xt", [D, T], F16, isOutput=False)
    wq = nc.declare_dram_parameter("wq", [D, E], F16, isOutput=False)
    wk = nc.declare_dram_parameter("wk", [D, E], F16, isOutput=False)
    wv = nc.declare_dram_parameter("wv", [D, E], F16, isOutput=False)
    wo = nc.declare_dram_parameter("wo", [E, D], F16, isOutput=False)
    if causal:
        em2 = nc.declare_dram_parameter("em2", [128, 2, 2, 512], F16, isOutput=False)
    else:
        em = nc.declare_dram_parameter("em", [T, T], F16, isOutput=False)
    ons = nc.declare_dram_parameter("ons", [128, 128], F16, isOutput=False)
    out = nc.declare_dram_parameter("out", [T, D], F16, isOutput=True)

    with tile.TileContext(nc) as tc:
        # ---- long-lived residents --------------------------------------
        pool_res = tc.alloc_tile_pool(name="res", bufs=1)
        ctx = [pool_res.tile([128, T], F16, name=f"ctx{m}") for m in range(NE)]
        v_sb = pool_res.tile([128, NT, E], F16)
        wo_sb = pool_res.tile([128, NE, D], F16)
        ones_sb = pool_res.tile([128, 128], F16)
        scratch = pool_res.tile([1, 8], F16)
        if causal:
            em_sb = pool_res.tile([128, 2, 2, 512], F16)

        pool_qk = tc.alloc_tile_pool(name="res_qk", bufs=1)
        qT = [pool_qk.tile([128, T], F16, name=f"qT{m}") for m in range(NE)]
        kT = [pool_qk.tile([128, T], F16, name=f"kT{m}") for m in range(NE)]

        pool_p1 = tc.alloc_tile_pool(name="p1", bufs=1)
        xt_sb = pool_p1.tile([128, ND, T], F16)
        wq_sb = pool_p1.tile([128, ND, E], F16)
        wk_sb = pool_p1.tile([128, ND, E], F16)
        wv_sb = pool_p1.tile([128, ND, E], F16)

        # ---- DMA schedule: wq+x(tch0) interleaved first so the first
        # projection pass can ride the stream, then the rest.
        for dt in range(ND):
            nc.sync.dma_start(out=wq_sb[:, dt, :], in_=wq.ap()[dt * 128:(dt + 1) * 128, :])
            nc.sync.dma_start(
                out=xt_sb[:, dt, 0:1024],
                in_=xt.ap()[dt * 128:(dt + 1) * 128, 0:1024])
        for dt in range(ND):
            nc.sync.dma_start(out=wk_sb[:, dt, :], in_=wk.ap()[dt * 128:(dt + 1) * 128, :])
        for dt in range(ND):
            nc.sync.dma_start(out=wv_sb[:, dt, :], in_=wv.ap()[dt * 128:(dt + 1) * 128, :])
        for dt in range(ND):
            nc.sync.dma_start(
                out=xt_sb[:, dt, 1024:2048],
                in_=xt.ap()[dt * 128:(dt + 1) * 128, 1024:2048])
        nc.sync.dma_start(out=ones_sb, in_=ons.ap())
        if causal:
            nc.sync.dma_start(out=em_sb, in_=em2.ap())
        for et in range(NE):
            nc.sync.dma_start(out=wo_sb[:, et, :], in_=wo.ap()[et * 128:(et + 1) * 128, :])

        # warm the ACT exp table set before P2 needs it
        nc.scalar.activation(scratch[0:1, 0:1], wq_sb[0:1, 0, 0:1], EXP)

        scope_p1 = nc.named_scope("P1_qkv"); scope_p1.__enter__()
        # ---- P1: q/k (feature-major) and v (token-major) projections ----
        p_psqk = tc.alloc_tile_pool(name="p1psqk", bufs=2, space="PSUM")
        p_psv = tc.alloc_tile_pool(name="p1psv", bufs=3, space="PSUM")
        for tch in range(NCH):
            for ti, (w_sb, dst) in enumerate(((wq_sb, qT), (wk_sb, kT))):
                for m in range(NE):
                    for half in range(2):
                        tsl = slice(tch * 1024 + half * 512,
                                    tch * 1024 + (half + 1) * 512)
                        ps = p_psqk.tile([128, 512], F32, name="ps_qk")
                        for dt in range(ND):
                            nc.tensor.matmul(ps, w_sb[:, dt, m * 128:(m + 1) * 128],
                                             xt_sb[:, dt, tsl],
                                             start=(dt == 0), stop=(dt == ND - 1))
                        if (m + ti + half) % 2 == 0:
                            nc.scalar.copy(dst[m][:, tsl], ps)
                        else:
                            nc.vector.tensor_copy(dst[m][:, tsl], ps)
            for tb in range(8):
                tbg = tch * 8 + tb
                ps = p_psv.tile([128, 512], F32, name="ps_v")
                for dt in range(ND):
                    nc.tensor.matmul(
                        ps, xt_sb[:, dt, tbg * 128:(tbg + 1) * 128],
                        wv_sb[:, dt, :], start=(dt == 0), stop=(dt == ND - 1))
                if tb % 2 == 0:
                    nc.vector.tensor_copy(v_sb[:, tbg, :], ps)
                else:
                    nc.scalar.copy(v_sb[:, tbg, :], ps)
        p_psv.release()
        p_psqk.release()
        pool_p1.release()
        scope_p1.__exit__(None, None, None)

        scope_p2 = nc.named_scope("P2_attn"); scope_p2.__enter__()
        # ---- P2: attention ---------------------------------------------
        p_pt = tc.alloc_tile_pool(name="p2pt", bufs=3)
        p_em = tc.alloc_tile_pool(name="p2em", bufs=3)
        p_bs = tc.alloc_tile_pool(name="p2bs", bufs=2)
        ps_s_pool = tc.alloc_tile_pool(name="p2pss", bufs=2, space="PSUM")
        ps_c_pool = tc.alloc_tile_pool(name="p2psc", bufs=2, space="PSUM")
        ps_l_pool = tc.alloc_tile_pool(name="p2psl", bufs=2, space="PSUM")

        for ic in range(NI):
            isl = slice(ic * 512, (ic + 1) * 512)
            surv = [jt for jt in range(NJ) if cls[jt, ic] != SKIP]
            assert surv, f"i-chunk {ic}: every key block masked"
            first, last = surv[0], surv[-1]
            pairs = [tuple(surv[i:i + 2]) for i in range(0, len(surv), 2)]
            for hp in range(HPC // 2):
                heads = (2 * hp, 2 * hp + 1)
                cps, lps = {}, {}
                for h in heads:
                    cps[h] = ps_c_pool.tile([128, 512], F32, name="ps_c", tag="c")
                    lps[h] = ps_l_pool.tile([128, 512], F32, name="ps_l", tag="l")
                prev = None
                for pr in pairs:
                    # query-range trim: the upper diagonal pair only attends
                    # to the last 256 queries of the chunk
                    if causal and cls[pr[0], ic] == MIXED and pr[0] == 4 * ic + 2:
                        qo, qn = 256, 256
                    else:
                        qo, qn = 0, 512
                    # mask operand (None / resident slice / DMA'd)
                    emop = None
                    if causal and cls[pr[0], ic] == MIXED:
                        # both diagonal pairs reduce to the (o0, o1) patterns
                        # over their query window
                        emop = em_sb[:, 0, :, 0:qn]
                    elif not causal and any(cls[jt, ic] == MIXED for jt in pr):
                        emt = p_em.tile([128, 2, 512], F16, name="emt")
                        for j, jt in enumerate(pr):
                            if cls[jt, ic] == MIXED:
                                nc.sync.dma_start(
                                    out=emt[:, j, :],
                                    in_=em.ap()[jt * 128:(jt + 1) * 128, isl])
                            else:
                                nc.vector.memset(emt[:, j, :], 1.0)
                        emop = emt[:, :, 0:qn]
                    pts = {}
                    for h in heads:
                        ps_s = ps_s_pool.tile([128, 2, 512], F32, name="ps_s")
                        for j, jt in enumerate(pr):
                            nc.tensor.matmul(
                                ps_s[:, j, 0:qn], kT[h][:, jt * 128:(jt + 1) * 128],
                                qT[h][:, ic * 512 + qo:ic * 512 + qo + qn],
                                start=True, stop=True)
                        pt = p_pt.tile([128, 2, 512], F16, name="pt")
                        if len(pr) == 2:
                            nc.scalar.activation(
                                pt[:, :, 0:qn], ps_s[:, :, 0:qn], EXP)
                        else:
                            nc.scalar.activation(
                                pt[:, 0, 0:qn], ps_s[:, 0, 0:qn], EXP)
                        if emop is not None:
                            if len(pr) == 2:
                                nc.vector.tensor_mul(
                                    pt[:, :, 0:qn], pt[:, :, 0:qn], emop)
                            else:
                                nc.vector.tensor_mul(
                                    pt[:, 0, 0:qn], pt[:, 0, 0:qn],
                                    emop[:, 0, :])
                        pts[h] = (pt, qo, qn)
                    if prev is not None:
                        self_pr, self_pts = prev
                        for h in heads:
                            ppt, pqo, pqn = self_pts[h]
                            psl = slice(pqo, pqo + pqn)
                            for j, jt in enumerate(self_pr):
                                st, sp = jt == first, jt == last
                                nc.tensor.matmul(
                                    cps[h][:, psl],
                                    v_sb[:, jt, h * 128:(h + 1) * 128],
                                    ppt[:, j, 0:pqn], start=st, stop=sp,
                                    skip_group_check=True)
                            for j, jt in enumerate(self_pr):
                                st, sp = jt == first, jt == last
                                nc.tensor.matmul(
                                    lps[h][:, psl], ones_sb,
                                    ppt[:, j, 0:pqn], start=st, stop=sp,
                                    skip_group_check=True)
                    prev = (pr, pts)
                self_pr, self_pts = prev
                for h in heads:
                    ppt, pqo, pqn = self_pts[h]
                    psl = slice(pqo, pqo + pqn)
                    for j, jt in enumerate(self_pr):
                        st, sp = jt == first, jt == last
                        nc.tensor.matmul(
                            cps[h][:, psl], v_sb[:, jt, h * 128:(h + 1) * 128],
                            ppt[:, j, 0:pqn], start=st, stop=sp,
                            skip_group_check=True)
                    for j, jt in enumerate(self_pr):
                        st, sp = jt == first, jt == last
                        nc.tensor.matmul(
                            lps[h][:, psl], ones_sb, ppt[:, j, 0:pqn],
                            start=st, stop=sp, skip_group_check=True)
                for h in heads:
                    bsb = p_bs.tile([128, 512], F16, name="bsb")
                    with nc.allow_low_precision(reason="softmax recip f16"):
                        nc.vector.reciprocal(bsb, lps[h])
                    nc.vector.tensor_mul(ctx[h][:, isl], cps[h], bsb)
        for p in (ps_l_pool, ps_c_pool, ps_s_pool, p_bs, p_em, p_pt):
            p.release()
        pool_qk.release()
        scope_p2.__exit__(None, None, None)

        scope_p3 = nc.named_scope("P3_out"); scope_p3.__enter__()
        # ---- P3: output projection -------------------------------------
        p_ot = tc.alloc_tile_pool(name="p3o", bufs=3)
        p_ps3 = tc.alloc_tile_pool(name="p3ps", bufs=2, space="PSUM")
        for tt in range(NT):
            tsl = slice(tt * 128, (tt + 1) * 128)
            for nch in range(NI):
                ps_o = p_ps3.tile([128, 512], F32, name="ps_o")
                for et in range(NE):
                    nc.tensor.matmul(
                        ps_o, ctx[et][:, tsl],
                        wo_sb[:, et, nch * 512:(nch + 1) * 512],
                        start=(et == 0), stop=(et == NE - 1))
                ot = p_ot.tile([128, 512], F16, name="ot")
                if (tt + nch) % 2 == 0:
                    nc.scalar.copy(ot, ps_o)
                else:
                    nc.vector.tensor_copy(ot, ps_o)
                nc.sync.dma_start(
                    out=out.ap()[tsl, nch * 512:(nch + 1) * 512], in_=ot)
        p_ps3.release()
        p_ot.release()
        pool_res.release()
        scope_p3.__exit__(None, None, None)

    nc.compile()
    return nc


def _get_nc(cls_key, causal):
    key = (cls_key, causal)
    if key not in _NC_CACHE:
        _NC_CACHE[key] = _build(cls_key, causal)
    return _NC_CACHE[key]


def _causal_pattern(o):
    p = np.arange(128)[:, None]
    f = np.arange(512)[None, :]
    return (p + o * 128 <= f).astype(np.float16)


def kernel(x, Wq, Wk, Wv, Wo, attn_mask):
    x = np.asarray(x, dtype=np.float32)
    Wq = np.asarray(Wq, dtype=np.float32)
    Wk = np.asarray(Wk, dtype=np.float32)
    Wv = np.asarray(Wv, dtype=np.float32)
    Wo = np.asarray(Wo, dtype=np.float32)
    mask = np.asarray(attn_mask, dtype=np.float32).reshape(T, T)

    emT = np.ascontiguousarray(np.exp(mask).T)
    scale = np.float32(1.0 / np.sqrt(DH))

    blocks = emT.reshape(NJ, 128, NI, 512)
    cls = np.full((NJ, NI), MIXED, dtype=np.int64)
    for jt in range(NJ):
        for ic in range(NI):
            sub = blocks[jt, :, ic, :]
            if not sub.any():
                cls[jt, ic] = SKIP
            elif np.all(sub == 1.0):
                cls[jt, ic] = NOMULT
    cls_key = tuple(cls.flatten().tolist())

    # causal fast path: survivors are a prefix, MIXED blocks are the last 4
    # of each i-chunk and match the canonical diagonal patterns
    causal = True
    pat = [_causal_pattern(o).astype(np.float32) for o in range(4)]
    for ic in range(NI):
        surv = [jt for jt in range(NJ) if cls[jt, ic] != SKIP]
        mix = [jt for jt in range(NJ) if cls[jt, ic] == MIXED]
        if surv != list(range(4 * ic + 4)) or mix != list(range(4 * ic, 4 * ic + 4)):
            causal = False
            break
        for jt in mix:
            if not np.array_equal(blocks[jt, :, ic, :], pat[jt - 4 * ic]):
                causal = False
                break
        if not causal:
            break

    em2 = np.zeros((128, 2, 2, 512), dtype=np.float16)
    for kk in range(2):
        for j in range(2):
            em2[:, kk, j, :] = _causal_pattern(2 * kk + j)

    xT = [np.ascontiguousarray(x[b].T).astype(np.float16) for b in range(B)]
    emT16 = emT.astype(np.float16)

    in_maps = []
    for c in range(8):
        b, g = c // 4, c % 4
        rows = slice(E * g, E * (g + 1))
        m = {
            "xt": xT[b],
            "wq": np.ascontiguousarray((Wq[rows, :] * scale).T).astype(np.float16),
            "wk": np.ascontiguousarray(Wk[rows, :].T).astype(np.float16),
            "wv": np.ascontiguousarray(Wv[rows, :].T).astype(np.float16),
            "wo": np.ascontiguousarray(Wo[:, rows].T).astype(np.float16),
            "ons": np.ones((128, 128), dtype=np.float16),
        }
        if causal:
            m["em2"] = em2
        else:
            m["em"] = emT16
        in_maps.append(m)

    global _LAST_IN_MAPS, _LAST_NC
    _LAST_IN_MAPS = in_maps
    nc = _get_nc(cls_key, causal)
    _LAST_NC = nc
    res = run_bass_kernel_spmd(nc, in_maps, list(range(8)))
    outs = [r["out"].astype(np.float32) for r in res.results]
    full = np.stack([
        outs[0] + outs[1] + outs[2] + outs[3],
        outs[4] + outs[5] + outs[6] + outs[7],
    ]).astype(np.float32)
    return full


# revision 11
# speedup vs baseline: 1.5719x; 1.1348x over previous
"""Fused multi-head attention (B=2, T=2048, D=2048, H=16) on 8 trn2 NeuronCores.

Sharding: core c handles batch b=c//4 and heads [4g, 4g+4), g=c%4 (tensor
parallel over heads x data parallel over batch). Each core computes its
4 heads' contribution to out[b] = attn(x[b]) @ Wo^T; the host sums the 4
partials per batch.

v2: all matmul operands fp16 (PSUM stays fp32), x^T DMA'd once into
resident SBUF, weights prefetched at kernel start, v computed directly in
[token, feature] layout (x-stationary matmuls, no PE transposes), exp
batched over j-tile pairs ([128,1024] ACT calls), causal diagonal masks as
2 resident [128,2,512] pattern tiles, software-pipelined attention inner
loop sized to exactly 8 PSUM banks, fp16 output.

Device algorithm (per core, E=512 features = 4 heads):
  P1  qT/kT = (W_s) @ x^T   [E rows as 4x(dh=128), T]   (Wq pre-scaled)
      v     = x @ Wv_s^T    [T, E]
  P2  per i-chunk (512 q), head pair: S^T pair = kT_jt^T-contract @ qT
        -> exp (ACT, [128,1024]) -> *mask (diag pairs) ->
        ctx^T += v_jt^T @ P^T ; L += ones128^T @ P^T (l replicated on all
        128 partitions, same PE tile config as ctx, no broadcast needed);
        ctx^T *= recip(L).  Upper-diagonal pairs trimmed to 256 queries.
  P3  out[t, d] = sum_e ctx^T[e, t] * WoT[e, d] -> DRAM (fp16)
"""

import numpy as np

import concourse.bass as bass
import concourse.mybir as mybir
import concourse.tile as tile
from concourse import bacc
from concourse.bass_utils import run_bass_kernel_spmd

F32 = mybir.dt.float32
F16 = mybir.dt.float16
EXP = mybir.ActivationFunctionType.Exp

B, T, D, H = 2, 2048, 2048, 16
DH = D // H          # 128
E = 512              # features per core (4 heads)
HPC = 4              # heads per core
NT = T // 128        # 16 token tiles
ND = D // 128        # 16 model-dim tiles
NE = E // 128        # 4 e-tiles per core
NI = T // 512        # 4 i-chunks (query chunks)
NJ = NT              # 16 j-tiles (key tiles)
NCH = T // 1024      # 2 big token chunks for the projections

_NC_CACHE = {}

# per-(jt, ic) mask-block class: 0 = fully masked (skip), 1 = unmasked
# (skip the mask multiply), 2 = mixed (multiply by exp(mask) elementwise)
SKIP, NOMULT, MIXED = 0, 1, 2


def _build(cls_key, causal):
    cls = np.asarray(cls_key, dtype=np.int64).reshape(NJ, NI)
    nc = bacc.Bacc(None, target_bir_lowering=False, debug=False)
    xt = nc.declare_dram_parameter("xt", [D, T], F16, isOutput=False)
    wq = nc.declare_dram_parameter("wq", [D, E], F16, isOutput=False)
    wk = nc.declare_dram_parameter("wk", [D, E], F16, isOutput=False)
    wv = nc.declare_dram_parameter("wv", [D, E], F16, isOutput=False)
    wo = nc.declare_dram_parameter("wo", [E, D], F16, isOutput=False)
    if causal:
        em2 = nc.declare_dram_parameter("em2", [128, 2, 2, 512], F16, isOutput=False)
    else:
        em = nc.declare_dram_parameter("em", [T, T], F16, isOutput=False)
    ons = nc.declare_dram_parameter("ons", [128, 128], F16, isOutput=False)
    out = nc.declare_dram_parameter("out", [T, D], F16, isOutput=True)

    with tile.TileContext(nc) as tc:
        # ---- long-lived residents --------------------------------------
        pool_res = tc.alloc_tile_pool(name="res", bufs=1)
        ctx = [pool_res.tile([128, T], F16, name=f"ctx{m}") for m in range(NE)]
        v_sb = pool_res.tile([128, NT, E], F16)
        wo_sb = pool_res.tile([128, NE, D], F16)
        ones_sb = pool_res.tile([128, 128], F16)
        scratch = pool_res.tile([1, 8], F16)
        if causal:
            em_sb = pool_res.tile([128, 2, 2, 512], F16)

        pool_qk = tc.alloc_tile_pool(name="res_qk", bufs=1)
        qT = [pool_qk.tile([128, T], F16, name=f"qT{m}") for m in range(NE)]
        kT = [pool_qk.tile([128, T], F16, name=f"kT{m}") for m in range(NE)]

        pool_p1 = tc.alloc_tile_pool(name="p1", bufs=1)
        xt_sb = pool_p1.tile([128, ND, T], F16)
        wq_sb = pool_p1.tile([128, ND, E], F16)
        wk_sb = pool_p1.tile([128, ND, E], F16)
        wv_sb = pool_p1.tile([128, ND, E], F16)

        # ---- DMA schedule: wq+x(tch0) interleaved first so the first
        # projection pass can ride the stream, then the rest.
        for dt in range(ND):
            nc.sync.dma_start(out=wq_sb[:, dt, :], in_=wq.ap()[dt * 128:(dt + 1) * 128, :])
            nc.sync.dma_start(
                out=xt_sb[:, dt, 0:1024],
                in_=xt.ap()[dt * 128:(dt + 1) * 128, 0:1024])
        for dt in range(ND):
            nc.sync.dma_start(out=wk_sb[:, dt, :], in_=wk.ap()[dt * 128:(dt + 1) * 128, :])
        for dt in range(ND):
            nc.sync.dma_start(out=wv_sb[:, dt, :], in_=wv.ap()[dt * 128:(dt + 1) * 128, :])
        for dt in range(ND):
            nc.sync.dma_start(
                out=xt_sb[:, dt, 1024:2048],
                in_=xt.ap()[dt * 128:(dt + 1) * 128, 1024:2048])
        nc.sync.dma_start(out=ones_sb, in_=ons.ap())
        if causal:
            nc.sync.dma_start(out=em_sb, in_=em2.ap())
        for et in range(NE):
            nc.sync.dma_start(out=wo_sb[:, et, :], in_=wo.ap()[et * 128:(et + 1) * 128, :])

        # warm the ACT exp table set before P2 needs it
        nc.scalar.activation(scratch[0:1, 0:1], wq_sb[0:1, 0, 0:1], EXP)

        scope_p1 = nc.named_scope("P1_qkv"); scope_p1.__enter__()
        # ---- P1: q/k (feature-major) and v (token-major) projections ----
        p_psqk = tc.alloc_tile_pool(name="p1psqk", bufs=2, space="PSUM")
        p_psv = tc.alloc_tile_pool(name="p1psv", bufs=3, space="PSUM")
        for tch in range(NCH):
            for ti, (w_sb, dst) in enumerate(((wq_sb, qT), (wk_sb, kT))):
                for m in range(NE):
                    for half in range(2):
                        tsl = slice(tch * 1024 + half * 512,
                                    tch * 1024 + (half + 1) * 512)
                        ps = p_psqk.tile([128, 512], F32, name="ps_qk")
                        for dt in range(ND):
                            nc.tensor.matmul(ps, w_sb[:, dt, m * 128:(m + 1) * 128],
                                             xt_sb[:, dt, tsl],
                                             start=(dt == 0), stop=(dt == ND - 1))
                        if (m + ti + half) % 2 == 0:
                            nc.scalar.copy(dst[m][:, tsl], ps)
                        else:
                            nc.vector.tensor_copy(dst[m][:, tsl], ps)
            for tb in range(8):
                tbg = tch * 8 + tb
                ps = p_psv.tile([128, 512], F32, name="ps_v")
                for dt in range(ND):
                    nc.tensor.matmul(
                        ps, xt_sb[:, dt, tbg * 128:(tbg + 1) * 128],
                        wv_sb[:, dt, :], start=(dt == 0), stop=(dt == ND - 1))
                if tb % 2 == 0:
                    nc.vector.tensor_copy(v_sb[:, tbg, :], ps)
                else:
                    nc.scalar.copy(v_sb[:, tbg, :], ps)
        p_psv.release()
        p_psqk.release()
        pool_p1.release()
        scope_p1.__exit__(None, None, None)

        scope_p2 = nc.named_scope("P2_attn"); scope_p2.__enter__()
        # ---- P2: attention ---------------------------------------------
        p_pt = tc.alloc_tile_pool(name="p2pt", bufs=3)
        p_em = tc.alloc_tile_pool(name="p2em", bufs=3)
        p_bs = tc.alloc_tile_pool(name="p2bs", bufs=2)
        ps_s_pool = tc.alloc_tile_pool(name="p2pss", bufs=2, space="PSUM")
        ps_c_pool = tc.alloc_tile_pool(name="p2psc", bufs=2, space="PSUM")
        ps_l_pool = tc.alloc_tile_pool(name="p2psl", bufs=2, space="PSUM")

        for ic in range(NI):
            isl = slice(ic * 512, (ic + 1) * 512)
            surv = [jt for jt in range(NJ) if cls[jt, ic] != SKIP]
            assert surv, f"i-chunk {ic}: every key block masked"
            first, last = surv[0], surv[-1]
            pairs = [tuple(surv[i:i + 2]) for i in range(0, len(surv), 2)]
            for hp in range(HPC // 2):
                heads = (2 * hp, 2 * hp + 1)
                cps, lps = {}, {}
                for h in heads:
                    cps[h] = ps_c_pool.tile([128, 512], F32, name="ps_c", tag="c")
                    lps[h] = ps_l_pool.tile([128, 512], F32, name="ps_l", tag="l")
                prev = None
                for pr in pairs:
                    # query-range trim: the upper diagonal pair only attends
                    # to the last 256 queries of the chunk
                    if causal and cls[pr[0], ic] == MIXED and pr[0] == 4 * ic + 2:
                        qo, qn = 256, 256
                    else:
                        qo, qn = 0, 512
                    # mask operand (None / resident slice / DMA'd)
                    emop = None
                    if causal and cls[pr[0], ic] == MIXED:
                        # both diagonal pairs reduce to the (o0, o1) patterns
                        # over their query window
                        emop = em_sb[:, 0, :, 0:qn]
                    elif not causal and any(cls[jt, ic] == MIXED for jt in pr):
                        emt = p_em.tile([128, 2, 512], F16, name="emt")
                        for j, jt in enumerate(pr):
                            if cls[jt, ic] == MIXED:
                                nc.sync.dma_start(
                                    out=emt[:, j, :],
                                    in_=em.ap()[jt * 128:(jt + 1) * 128, isl])
                            else:
                                nc.vector.memset(emt[:, j, :], 1.0)
                        emop = emt[:, :, 0:qn]
                    pts = {}
                    for h in heads:
                        ps_s = ps_s_pool.tile([128, 2, 512], F32, name="ps_s")
                        for j, jt in enumerate(pr):
                            nc.tensor.matmul(
                                ps_s[:, j, 0:qn], kT[h][:, jt * 128:(jt + 1) * 128],
                                qT[h][:, ic * 512 + qo:ic * 512 + qo + qn],
                                start=True, stop=True)
                        pt = p_pt.tile([128, 2, 512], F16, name="pt")
                        if len(pr) == 2:
                            nc.scalar.activation(
                                pt[:, :, 0:qn], ps_s[:, :, 0:qn], EXP)
                        else:
                            nc.scalar.activation(
                                pt[:, 0, 0:qn], ps_s[:, 0, 0:qn], EXP)
                        if emop is not None:
                            if len(pr) == 2:
                                nc.vector.tensor_mul(
                                    pt[:, :, 0:qn], pt[:, :, 0:qn], emop)
                            else:
                                nc.vector.tensor_mul(
                                    pt[:, 0, 0:qn], pt[:, 0, 0:qn],
                                    emop[:, 0, :])
                        pts[h] = (pt, qo, qn)
                    if prev is not None:
                        self_pr, self_pts = prev
                        for h in heads:
                            ppt, pqo, pqn = self_pts[h]
                            psl = slice(pqo, pqo + pqn)
                            for j, jt in enumerate(self_pr):
                                st, sp = jt == first, jt == last
                                nc.tensor.matmul(
                                    cps[h][:, psl],
                                    v_sb[:, jt, h * 128:(h + 1) * 128],
                                    ppt[:, j, 0:pqn], start=st, stop=sp,
                                    skip_group_check=True)
                            for j, jt in enumerate(self_pr):
                                st, sp = jt == first, jt == last
                                nc.tensor.matmul(
                                    lps[h][:, psl], ones_sb,
                                    ppt[:, j, 0:pqn], start=st, stop=sp,
                                    skip_group_check=True)
                    prev = (pr, pts)
                self_pr, self_pts = prev
                for h in heads:
                    ppt, pqo, pqn = self_pts[h]
                    psl = slice(pqo, pqo + pqn)
                    for j, jt in enumerate(self_pr):
                        st, sp = jt == first, jt == last
                        nc.tensor.matmul(
                            cps[h][:, psl], v_sb[:, jt, h * 128:(h + 1) * 128],
                            ppt[:, j, 0:pqn], start=st, stop=sp,
                            skip_group_check=True)
                    for j, jt in enumerate(self_pr):
                        st, sp = jt == first, jt == last
                        nc.tensor.matmul(
                            lps[h][:, psl], ones_sb, ppt[:, j, 0:pqn],
                            start=st, stop=sp, skip_group_check=True)
                for h in heads:
                    bsb = p_bs.tile([128, 512], F32, name="bsb")
                    nc.vector.reciprocal_approx_fast(out=bsb, in_=lps[h])
                    nc.vector.tensor_mul(ctx[h][:, isl], cps[h], bsb)
        for p in (ps_l_pool, ps_c_pool, ps_s_pool, p_bs, p_em, p_pt):
            p.release()
        pool_qk.release()
        scope_p2.__exit__(None, None, None)

        scope_p3 = nc.named_scope("P3_out"); scope_p3.__enter__()
        # ---- P3: output projection -------------------------------------
        p_ot = tc.alloc_tile_pool(name="p3o", bufs=3)
        p_ps3 = tc.alloc_tile_pool(name="p3ps", bufs=2, space="PSUM")
        for tt in range(NT):
            tsl = slice(tt * 128, (tt + 1) * 128)
            for nch in range(NI):
                ps_o = p_ps3.tile([128, 512], F32, name="ps_o")
                for et in range(NE):
                    nc.tensor.matmul(
                        ps_o, ctx[et][:, tsl],
                        wo_sb[:, et, nch * 512:(nch + 1) * 512],
                        start=(et == 0), stop=(et == NE - 1))
                ot = p_ot.tile([128, 512], F16, name="ot")
                if (tt + nch) % 2 == 0:
                    nc.scalar.copy(ot, ps_o)
                else:
                    nc.vector.tensor_copy(ot, ps_o)
                nc.sync.dma_start(
                    out=out.ap()[tsl, nch * 512:(nch + 1) * 512], in_=ot)
        p_ps3.release()
        p_ot.release()
        pool_res.release()
        scope_p3.__exit__(None, None, None)

    nc.compile()
    return nc


def _get_nc(cls_key, causal):
    key = (cls_key, causal)
    if key not in _NC_CACHE:
        _NC_CACHE[key] = _build(cls_key, causal)
    return _NC_CACHE[key]


def _causal_pattern(o):
    p = np.arange(128)[:, None]
    f = np.arange(512)[None, :]
    return (p + o * 128 <= f).astype(np.float16)


def kernel(x, Wq, Wk, Wv, Wo, attn_mask):
    x = np.asarray(x, dtype=np.float32)
    Wq = np.asarray(Wq, dtype=np.float32)
    Wk = np.asarray(Wk, dtype=np.float32)
    Wv = np.asarray(Wv, dtype=np.float32)
    Wo = np.asarray(Wo, dtype=np.float32)
    mask = np.asarray(attn_mask, dtype=np.float32).reshape(T, T)

    emT = np.ascontiguousarray(np.exp(mask).T)
    scale = np.float32(1.0 / np.sqrt(DH))

    blocks = emT.reshape(NJ, 128, NI, 512)
    cls = np.full((NJ, NI), MIXED, dtype=np.int64)
    for jt in range(NJ):
        for ic in range(NI):
            sub = blocks[jt, :, ic, :]
            if not sub.any():
                cls[jt, ic] = SKIP
            elif np.all(sub == 1.0):
                cls[jt, ic] = NOMULT
    cls_key = tuple(cls.flatten().tolist())

    # causal fast path: survivors are a prefix, MIXED blocks are the last 4
    # of each i-chunk and match the canonical diagonal patterns
    causal = True
    pat = [_causal_pattern(o).astype(np.float32) for o in range(4)]
    for ic in range(NI):
        surv = [jt for jt in range(NJ) if cls[jt, ic] != SKIP]
        mix = [jt for jt in range(NJ) if cls[jt, ic] == MIXED]
        if surv != list(range(4 * ic + 4)) or mix != list(range(4 * ic, 4 * ic + 4)):
            causal = False
            break
        for jt in mix:
            if not np.array_equal(blocks[jt, :, ic, :], pat[jt - 4 * ic]):
                causal = False
                break
        if not causal:
            break

    em2 = np.zeros((128, 2, 2, 512), dtype=np.float16)
    for kk in range(2):
        for j in range(2):
            em2[:, kk, j, :] = _causal_pattern(2 * kk + j)

    xT = [np.ascontiguousarray(x[b].T).astype(np.float16) for b in range(B)]
    emT16 = emT.astype(np.float16)

    in_maps = []
    for c in range(8):
        b, g = c // 4, c % 4
        rows = slice(E * g, E * (g + 1))
        m = {
            "xt": xT[b],
            "wq": np.ascontiguousarray((Wq[rows, :] * scale).T).astype(np.float16),
            "wk": np.ascontiguousarray(Wk[rows, :].T).astype(np.float16),
            "wv": np.ascontiguousarray(Wv[rows, :].T).astype(np.float16),
            "wo": np.ascontiguousarray(Wo[:, rows].T).astype(np.float16),
            "ons": np.ones((128, 128), dtype=np.float16),
        }
        if causal:
            m["em2"] = em2
        else:
            m["em"] = emT16
        in_maps.append(m)

    global _LAST_IN_MAPS, _LAST_NC
    _LAST_IN_MAPS = in_maps
    nc = _get_nc(cls_key, causal)
    _LAST_NC = nc
    res = run_bass_kernel_spmd(nc, in_maps, list(range(8)))
    outs = [r["out"].astype(np.float32) for r in res.results]
    full = np.stack([
        outs[0] + outs[1] + outs[2] + outs[3],
        outs[4] + outs[5] + outs[6] + outs[7],
    ]).astype(np.float32)
    return full


# revision 17
# speedup vs baseline: 1.6752x; 1.0657x over previous
"""Fused multi-head attention (B=2, T=2048, D=2048, H=16) on 8 trn2 NeuronCores.

Sharding: core c handles batch b=c//4 and heads [4g, 4g+4), g=c%4 (tensor
parallel over heads x data parallel over batch). Each core computes its
4 heads' contribution to out[b] = attn(x[b]) @ Wo^T; the host sums the 4
partials per batch.

v2: all matmul operands fp16 (PSUM stays fp32), x^T DMA'd once into
resident SBUF, weights prefetched at kernel start, v computed directly in
[token, feature] layout (x-stationary matmuls, no PE transposes), exp
batched over j-tile pairs ([128,1024] ACT calls), causal diagonal masks as
2 resident [128,2,512] pattern tiles, software-pipelined attention inner
loop sized to exactly 8 PSUM banks, fp16 output.

Device algorithm (per core, E=512 features = 4 heads):
  P1  qT/kT = (W_s) @ x^T   [E rows as 4x(dh=128), T]   (Wq pre-scaled)
      v     = x @ Wv_s^T    [T, E]
  P2  per i-chunk (512 q), head pair: S^T pair = kT_jt^T-contract @ qT
        -> exp (ACT, [128,1024]) -> *mask (diag pairs) ->
        ctx^T += v_jt^T @ P^T ; L += ones128^T @ P^T (l replicated on all
        128 partitions, same PE tile config as ctx, no broadcast needed);
        ctx^T *= recip(L).  Upper-diagonal pairs trimmed to 256 queries.
  P3  out[t, d] = sum_e ctx^T[e, t] * WoT[e, d] -> DRAM (fp16)
"""

import numpy as np

import concourse.bass as bass
import concourse.mybir as mybir
import concourse.tile as tile
from concourse import bacc
from concourse.bass_utils import run_bass_kernel_spmd

F32 = mybir.dt.float32
F16 = mybir.dt.float16
EXP = mybir.ActivationFunctionType.Exp

B, T, D, H = 2, 2048, 2048, 16
DH = D // H          # 128
E = 512              # features per core (4 heads)
HPC = 4              # heads per core
NT = T // 128        # 16 token tiles
ND = D // 128        # 16 model-dim tiles
NE = E // 128        # 4 e-tiles per core
NI = T // 512        # 4 i-chunks (query chunks)
NJ = NT              # 16 j-tiles (key tiles)
NCH = T // 1024      # 2 big token chunks for the projections

_NC_CACHE = {}

# per-(jt, ic) mask-block class: 0 = fully masked (skip), 1 = unmasked
# (skip the mask multiply), 2 = mixed (multiply by exp(mask) elementwise)
SKIP, NOMULT, MIXED = 0, 1, 2


def _build(cls_key, causal):
    cls = np.asarray(cls_key, dtype=np.int64).reshape(NJ, NI)
    nc = bacc.Bacc(None, target_bir_lowering=False, debug=False)
    # all staged operands are host-permuted to SBUF layout [128, tile, free]
    # so each one is a single large DMA
    xt = nc.declare_dram_parameter("xt", [128, ND, T], F16, isOutput=False)
    wq = nc.declare_dram_parameter("wq", [128, ND, E], F16, isOutput=False)
    wk = nc.declare_dram_parameter("wk", [128, ND, E], F16, isOutput=False)
    wv = nc.declare_dram_parameter("wv", [128, ND, E], F16, isOutput=False)
    wo = nc.declare_dram_parameter("wo", [128, NE, D], F16, isOutput=False)
    if causal:
        em2 = nc.declare_dram_parameter("em2", [128, 2, 2, 512], F16, isOutput=False)
    else:
        em = nc.declare_dram_parameter("em", [T, T], F16, isOutput=False)
    ons = nc.declare_dram_parameter("ons", [128, 128], F16, isOutput=False)
    out = nc.declare_dram_parameter("out", [T, D], F16, isOutput=True)

    with tile.TileContext(nc) as tc:
        # ---- long-lived residents --------------------------------------
        pool_res = tc.alloc_tile_pool(name="res", bufs=1)
        ctx = [pool_res.tile([128, T], F16, name=f"ctx{m}") for m in range(NE)]
        v_sb = pool_res.tile([128, NT, E], F16)
        wo_sb = pool_res.tile([128, NE, D], F16)
        ones_sb = pool_res.tile([128, 128], F16)
        scratch = pool_res.tile([1, 8], F16)
        if causal:
            em_sb = pool_res.tile([128, 2, 2, 512], F16)

        pool_qk = tc.alloc_tile_pool(name="res_qk", bufs=1)
        qT = [pool_qk.tile([128, T], F16, name=f"qT{m}") for m in range(NE)]
        kT = [pool_qk.tile([128, T], F16, name=f"kT{m}") for m in range(NE)]

        pool_p1 = tc.alloc_tile_pool(name="p1", bufs=1)
        xt_sb = pool_p1.tile([128, ND, T], F16)
        wq_sb = pool_p1.tile([128, ND, E], F16)
        wk_sb = pool_p1.tile([128, ND, E], F16)
        wv_sb = pool_p1.tile([128, ND, E], F16)

        # ---- DMA schedule: wv + per-token-block x first so the v-pass can
        # start after ~2.5MB instead of gating on the full 6MB q-projection
        # working set; the rest streams underneath the v-pass compute.
        nc.sync.dma_start(out=wv_sb, in_=wv.ap())
        for tb in range(8):
            tsl = slice(tb * 128, (tb + 1) * 128)
            nc.sync.dma_start(out=xt_sb[:, :, tsl], in_=xt.ap()[:, :, tsl])
        nc.sync.dma_start(out=xt_sb[:, :, 1024:1536], in_=xt.ap()[:, :, 1024:1536])
        nc.sync.dma_start(out=xt_sb[:, :, 1536:2048], in_=xt.ap()[:, :, 1536:2048])
        nc.sync.dma_start(out=wq_sb, in_=wq.ap())
        nc.sync.dma_start(out=wk_sb, in_=wk.ap())
        nc.sync.dma_start(out=ones_sb, in_=ons.ap())
        if causal:
            nc.sync.dma_start(out=em_sb, in_=em2.ap())
        nc.sync.dma_start(out=wo_sb, in_=wo.ap())

        # warm the ACT exp table set before P2 needs it
        nc.scalar.activation(scratch[0:1, 0:1], wv_sb[0:1, 0, 0:1], EXP)

        scope_p1 = nc.named_scope("P1_qkv"); scope_p1.__enter__()
        # ---- P1: v (token-major) first, then q/k (feature-major) --------
        p_psv = tc.alloc_tile_pool(name="p1psv", bufs=3, space="PSUM")
        p_psqk = tc.alloc_tile_pool(name="p1psqk", bufs=2, space="PSUM")
        for tbg in range(NT):
            ps = p_psv.tile([128, 512], F32, name="ps_v")
            for dt in range(ND):
                nc.tensor.matmul(
                    ps, xt_sb[:, dt, tbg * 128:(tbg + 1) * 128],
                    wv_sb[:, dt, :], start=(dt == 0), stop=(dt == ND - 1))
            if tbg % 2 == 0:
                nc.vector.tensor_copy(v_sb[:, tbg, :], ps)
            else:
                nc.scalar.copy(v_sb[:, tbg, :], ps)
        for tch in range(NCH):
            for ti, (w_sb, dst) in enumerate(((wq_sb, qT), (wk_sb, kT))):
                for m in range(NE):
                    for half in range(2):
                        tsl = slice(tch * 1024 + half * 512,
                                    tch * 1024 + (half + 1) * 512)
                        ps = p_psqk.tile([128, 512], F32, name="ps_qk")
                        for dt in range(ND):
                            nc.tensor.matmul(ps, w_sb[:, dt, m * 128:(m + 1) * 128],
                                             xt_sb[:, dt, tsl],
                                             start=(dt == 0), stop=(dt == ND - 1))
                        if (m + ti + half) % 2 == 0:
                            nc.scalar.copy(dst[m][:, tsl], ps)
                        else:
                            nc.vector.tensor_copy(dst[m][:, tsl], ps)
        p_psqk.release()
        p_psv.release()
        pool_p1.release()
        scope_p1.__exit__(None, None, None)

        scope_p2 = nc.named_scope("P2_attn"); scope_p2.__enter__()
        # ---- P2: attention ---------------------------------------------
        p_pt = tc.alloc_tile_pool(name="p2pt", bufs=3)
        p_em = tc.alloc_tile_pool(name="p2em", bufs=3)
        p_bs = tc.alloc_tile_pool(name="p2bs", bufs=2)
        ps_s_pool = tc.alloc_tile_pool(name="p2pss", bufs=2, space="PSUM")
        ps_c_pool = tc.alloc_tile_pool(name="p2psc", bufs=2, space="PSUM")
        ps_l_pool = tc.alloc_tile_pool(name="p2psl", bufs=2, space="PSUM")

        for ic in range(NI):
            isl = slice(ic * 512, (ic + 1) * 512)
            surv = [jt for jt in range(NJ) if cls[jt, ic] != SKIP]
            assert surv, f"i-chunk {ic}: every key block masked"
            first, last = surv[0], surv[-1]
            pairs = [tuple(surv[i:i + 2]) for i in range(0, len(surv), 2)]
            for h in range(HPC):
                cps = ps_c_pool.tile([128, 512], F32, name="ps_c", tag="c")
                lps = ps_l_pool.tile([128, 512], F32, name="ps_l", tag="l")

                def flush(prev):
                    ppt, ppr, pqo, pqn = prev
                    psl = slice(pqo, pqo + pqn)
                    for j, jt in enumerate(ppr):
                        st, sp = jt == first, jt == last
                        nc.tensor.matmul(
                            cps[:, psl], v_sb[:, jt, h * 128:(h + 1) * 128],
                            ppt[:, j, 0:pqn], start=st, stop=sp,
                            skip_group_check=True)
                    for j, jt in enumerate(ppr):
                        st, sp = jt == first, jt == last
                        nc.tensor.matmul(
                            lps[:, psl], ones_sb, ppt[:, j, 0:pqn],
                            start=st, stop=sp, skip_group_check=True)

                prev = None
                for pr in pairs:
                    # query-range trim: the upper diagonal pair only attends
                    # to the last 256 queries of the chunk
                    if causal and cls[pr[0], ic] == MIXED and pr[0] == 4 * ic + 2:
                        qo, qn = 256, 256
                    else:
                        qo, qn = 0, 512
                    # mask operand (None / resident slice / DMA'd)
                    emop = None
                    if causal and cls[pr[0], ic] == MIXED:
                        # both diagonal pairs reduce to the (o0, o1) patterns
                        # over their query window
                        emop = em_sb[:, 0, :, 0:qn]
                    elif not causal and any(cls[jt, ic] == MIXED for jt in pr):
                        emt = p_em.tile([128, 2, 512], F16, name="emt")
                        for j, jt in enumerate(pr):
                            if cls[jt, ic] == MIXED:
                                nc.sync.dma_start(
                                    out=emt[:, j, :],
                                    in_=em.ap()[jt * 128:(jt + 1) * 128, isl])
                            else:
                                nc.vector.memset(emt[:, j, :], 1.0)
                        emop = emt[:, :, 0:qn]
                    ps_s = ps_s_pool.tile([128, 2, 512], F32, name="ps_s")
                    for j, jt in enumerate(pr):
                        nc.tensor.matmul(
                            ps_s[:, j, 0:qn], kT[h][:, jt * 128:(jt + 1) * 128],
                            qT[h][:, ic * 512 + qo:ic * 512 + qo + qn],
                            start=True, stop=True)
                    pt = p_pt.tile([128, 2, 512], F16, name="pt")
                    if len(pr) == 2:
                        nc.scalar.activation(
                            pt[:, :, 0:qn], ps_s[:, :, 0:qn], EXP)
                    else:
                        nc.scalar.activation(
                            pt[:, 0, 0:qn], ps_s[:, 0, 0:qn], EXP)
                    if emop is not None:
                        if len(pr) == 2:
                            nc.vector.tensor_mul(
                                pt[:, :, 0:qn], pt[:, :, 0:qn], emop)
                        else:
                            nc.vector.tensor_mul(
                                pt[:, 0, 0:qn], pt[:, 0, 0:qn], emop[:, 0, :])
                    if prev is not None:
                        flush(prev)
                    prev = (pt, pr, qo, qn)
                flush(prev)
                bsb = p_bs.tile([128, 512], F32, name="bsb")
                nc.vector.reciprocal_approx_fast(out=bsb, in_=lps)
                nc.vector.tensor_mul(ctx[h][:, isl], cps, bsb)
        for p in (ps_l_pool, ps_c_pool, ps_s_pool, p_bs, p_em, p_pt):
            p.release()
        pool_qk.release()
        scope_p2.__exit__(None, None, None)

        scope_p3 = nc.named_scope("P3_out"); scope_p3.__enter__()
        # ---- P3: output projection -------------------------------------
        p_ot = tc.alloc_tile_pool(name="p3o", bufs=4)
        p_ps3 = tc.alloc_tile_pool(name="p3ps", bufs=4, space="PSUM")
        for tt in range(NT):
            tsl = slice(tt * 128, (tt + 1) * 128)
            for nch in range(NI):
                ps_o = p_ps3.tile([128, 512], F32, name="ps_o")
                for et in range(NE):
                    nc.tensor.matmul(
                        ps_o, ctx[et][:, tsl],
                        wo_sb[:, et, nch * 512:(nch + 1) * 512],
                        start=(et == 0), stop=(et == NE - 1))
                ot = p_ot.tile([128, 512], F16, name="ot")
                if (tt + nch) % 2 == 0:
                    nc.scalar.copy(ot, ps_o)
                else:
                    nc.vector.tensor_copy(ot, ps_o)
                nc.sync.dma_start(
                    out=out.ap()[tsl, nch * 512:(nch + 1) * 512], in_=ot)
        p_ps3.release()
        p_ot.release()
        pool_res.release()
        scope_p3.__exit__(None, None, None)

    nc.compile()
    return nc


def _get_nc(cls_key, causal):
    key = (cls_key, causal)
    if key not in _NC_CACHE:
        _NC_CACHE[key] = _build(cls_key, causal)
    return _NC_CACHE[key]


def _causal_pattern(o):
    p = np.arange(128)[:, None]
    f = np.arange(512)[None, :]
    return (p + o * 128 <= f).astype(np.float16)


def kernel(x, Wq, Wk, Wv, Wo, attn_mask):
    x = np.asarray(x, dtype=np.float32)
    Wq = np.asarray(Wq, dtype=np.float32)
    Wk = np.asarray(Wk, dtype=np.float32)
    Wv = np.asarray(Wv, dtype=np.float32)
    Wo = np.asarray(Wo, dtype=np.float32)
    mask = np.asarray(attn_mask, dtype=np.float32).reshape(T, T)

    emT = np.ascontiguousarray(np.exp(mask).T)
    scale = np.float32(1.0 / np.sqrt(DH))

    blocks = emT.reshape(NJ, 128, NI, 512)
    cls = np.full((NJ, NI), MIXED, dtype=np.int64)
    for jt in range(NJ):
        for ic in range(NI):
            sub = blocks[jt, :, ic, :]
            if not sub.any():
                cls[jt, ic] = SKIP
            elif np.all(sub == 1.0):
                cls[jt, ic] = NOMULT
    cls_key = tuple(cls.flatten().tolist())

    # causal fast path: survivors are a prefix, MIXED blocks are the last 4
    # of each i-chunk and match the canonical diagonal patterns
    causal = True
    pat = [_causal_pattern(o).astype(np.float32) for o in range(4)]
    for ic in range(NI):
        surv = [jt for jt in range(NJ) if cls[jt, ic] != SKIP]
        mix = [jt for jt in range(NJ) if cls[jt, ic] == MIXED]
        if surv != list(range(4 * ic + 4)) or mix != list(range(4 * ic, 4 * ic + 4)):
            causal = False
            break
        for jt in mix:
            if not np.array_equal(blocks[jt, :, ic, :], pat[jt - 4 * ic]):
                causal = False
                break
        if not causal:
            break

    em2 = np.zeros((128, 2, 2, 512), dtype=np.float16)
    for kk in range(2):
        for j in range(2):
            em2[:, kk, j, :] = _causal_pattern(2 * kk + j)

    def _perm(a, ntile):
        # [ntile*128, F] -> [128, ntile, F] contiguous fp16
        f = a.shape[1]
        return np.ascontiguousarray(
            a.reshape(ntile, 128, f).transpose(1, 0, 2)).astype(np.float16)

    xT = [_perm(x[b].T, ND) for b in range(B)]
    emT16 = emT.astype(np.float16)

    in_maps = []
    for c in range(8):
        b, g = c // 4, c % 4
        rows = slice(E * g, E * (g + 1))
        m = {
            "xt": xT[b],
            "wq": _perm((Wq[rows, :] * scale).T, ND),
            "wk": _perm(Wk[rows, :].T, ND),
            "wv": _perm(Wv[rows, :].T, ND),
            "wo": _perm(Wo[:, rows].T, NE),
            "ons": np.ones((128, 128), dtype=np.float16),
        }
        if causal:
            m["em2"] = em2
        else:
            m["em"] = emT16
        in_maps.append(m)

    global _LAST_IN_MAPS, _LAST_NC
    _LAST_IN_MAPS = in_maps
    nc = _get_nc(cls_key, causal)
    _LAST_NC = nc
    res = run_bass_kernel_spmd(nc, in_maps, list(range(8)))
    outs = [r["out"].astype(np.float32) for r in res.results]
    full = np.stack([
        outs[0] + outs[1] + outs[2] + outs[3],
        outs[4] + outs[5] + outs[6] + outs[7],
    ]).astype(np.float32)
    return full
